# revision 12
# baseline (speedup 1.0000x reference)
"""CPDecoding (embedding_lookup) Trainium2 kernel, v4.

out[n] = sum_c fz[c,n]*fy[c,n]*fx[c,n], where f* is a 1-D linear
interpolation (grid_sample, align_corners=True) of a (96, 512) line table
at per-point coordinates.

The kernel is DMA-descriptor-bound: every gather descriptor costs
max(elem_bytes*(2 if <512B else 1)/22.5, 7)/16 ns and elem_bytes must be a
multiple of 256, so each per-point fetch costs 22.76/16 ns no matter the
dtype. Strategy (8 cores, data-parallel over the N=4096*192 points):
  - Host: compute (i0, w) per point/axis; sort points by z-index and pack
    8 points per z-table row (one 512B descriptor serves 8 points), padded
    to a fixed slot count.
  - y AND x tables are 64x supersampled fp16 f0-only 256B rows
    (interpolation baked in): one descriptor per point per axis, and the
    x-interp disappears from the device (vs v2: -2 DVE ops, -1 Act op).
  - idx tensors are stored unreplicated [16, n] (the DMA gather only reads
    idx partitions 0:16) and streamed per chunk: 1/8th the idx DMA of v2,
    overlapped with gathers instead of an 11us serial prologue.
  - The last 256 f-slots run as 4 small chunks (1024 pts) so the trailing
    compute after the final gather is ~2us instead of ~9us.
  - Per 4096-point chunk: DMA ~12.5us (bound), DVE ~8.7us, Act ~2.8us,
    Pool ~6.1us, all overlapped.
  - Host: unpermute per-point sums to the original order.
y+x supersampling adds ~1.1% rel err (budget 2e-2).
"""

import numpy as np

N_CORES = 8
N_TOTAL = 4096 * 192
N_CORE = N_TOTAL // N_CORES      # 98304 points per core
P = 128                          # partitions
G = 8                            # points per z-row group
F = 800                          # free slots per partition (padded)
S = P * F                        # 102400 padded point slots per core
C = 96                           # components
R = 512                          # table resolution
ELEMZ = 256                      # fp16 elements per z table row (512 bytes)
SS = 64                          # y/x table supersampling factor
SELEM = 128                      # fp16 elements per y/x table row (256 bytes)
# chunk schedule: (f_start, chunk_f); small tail chunks shrink the final
# non-overlapped compute stretch
CHUNKS = [(32 * c, 32) for c in range(24)] + [(768 + 8 * k, 8) for k in range(4)]

_BUILT = None


def _build_nc():
    """Build the per-core Bass program (SPMD, identical on all cores)."""
    import concourse.bacc as bacc
    import concourse.tile as tile
    from concourse import mybir
    from concourse.library_config import mlp as lib_mlp

    dt = mybir.dt
    Axis = mybir.AxisListType

    nc = bacc.Bacc("TRN2", target_bir_lowering=False, debug=False,
                   num_devices=N_CORES, num_swdge_queues=1)

    # host-prepared inputs (idx wrapped-16 and replicated to all 8 bands --
    # each SWDGE engine reads its own 16-partition band)
    w_d = nc.dram_tensor("w", [P, F], dt.float16, kind="ExternalInput").ap()
    idxz_d = nc.dram_tensor("idxz", [P, F], dt.int16,
                            kind="ExternalInput").ap()
    idxy_d = nc.dram_tensor("idxy", [P, S // 16], dt.int16,
                            kind="ExternalInput").ap()
    idxx_d = nc.dram_tensor("idxx", [P, S // 16], dt.int16,
                            kind="ExternalInput").ap()
    tblz = nc.dram_tensor("tblz", [R, ELEMZ], dt.float16,
                          kind="ExternalInput").ap()
    tbly = nc.dram_tensor("tbly", [R * SS, SELEM], dt.float16,
                          kind="ExternalInput").ap()
    tblx = nc.dram_tensor("tblx", [R * SS, SELEM], dt.float16,
                          kind="ExternalInput").ap()
    out_d = nc.dram_tensor("out", [P, F], dt.float32, kind="ExternalOutput").ap()

    with tile.TileContext(nc) as tc:
        with tc.tile_pool(name="persist", bufs=1) as pp:
            w_all = pp.tile([P, F], dt.float16, tag="w")
            nc.sync.dma_start(w_all[:], w_d)
            idx_z = pp.tile([P, F], dt.int16, tag="iz")
            nc.sync.dma_start(idx_z[:], idxz_d)
            out_full = pp.tile([P, F], dt.float32, tag="out")

            with (
                tc.tile_pool(name="idx", bufs=2) as ip,
                tc.tile_pool(name="gath", bufs=2) as gp,
                tc.tile_pool(name="work", bufs=2) as wp,
            ):
                with tc.tile_critical():
                    nc.gpsimd.load_library(lib_mlp)
                for ci, (fs, cf) in enumerate(CHUNKS):
                    npts = P * cf                    # 4096 or 1024
                    ycols = npts // 16               # idx columns (wrapped-16)
                    nz = npts // G                   # z descriptors
                    zcols = nz // 16                 # == cf

                    # --- idx slices, streamed per chunk (overlaps gathers) ---
                    iy_t = ip.tile([P, 256], dt.int16, tag="iy")
                    nc.sync.dma_start(iy_t[:, 0:ycols],
                                      idxy_d[:, fs * 8:fs * 8 + ycols])
                    ix_t = ip.tile([P, 256], dt.int16, tag="ix")
                    nc.sync.dma_start(ix_t[:, 0:ycols],
                                      idxx_d[:, fs * 8:fs * 8 + ycols])

                    # --- gathers (x, y first; z is fast and cheap) ---
                    gx = gp.tile([P, 32, SELEM], dt.float16, tag="gx")
                    nc.gpsimd.dma_gather(
                        gx[:, 0:cf, :], tblx, ix_t[:, 0:ycols],
                        npts, npts, SELEM, elem_step=SELEM,
                        queue_num=0, single_packet=False)
                    gy = gp.tile([P, 32, SELEM], dt.float16, tag="gy")
                    nc.gpsimd.dma_gather(
                        gy[:, 0:cf, :], tbly, iy_t[:, 0:ycols],
                        npts, npts, SELEM, elem_step=SELEM,
                        queue_num=0, single_packet=False)
                    gz = gp.tile([P, 4, ELEMZ], dt.float16, tag="gz")
                    nc.gpsimd.dma_gather(
                        gz[:, 0:nz // P, :], tblz,
                        idx_z[:, fs:fs + zcols],
                        nz, nz, ELEMZ, elem_step=ELEMZ,
                        queue_num=0, single_packet=False)

                    # --- z weight broadcast (Act engine) ---
                    wtz = wp.tile([P, 32, C], dt.float16, tag="wtz")
                    nc.scalar.copy(wtz[:, 0:cf, :], w_all[:, fs:fs + cf]
                                   .unsqueeze(2).broadcast_to([P, cf, C]))

                    # --- z interp (rows shared by groups of 8 points) ---
                    zb = nz // P                     # 4 or 1
                    d_z = (gz[:, 0:zb, C:2 * C].unsqueeze(2)
                           .broadcast_to([P, zb, G, C]))
                    f0_z = (gz[:, 0:zb, 0:C].unsqueeze(2)
                            .broadcast_to([P, zb, G, C]))
                    fz = wp.tile([P, 32, C], dt.float16, tag="fz")
                    fz4 = fz[:, 0:cf, :].rearrange("p (q g) v -> p q g v", g=G)
                    wt4 = wtz[:, 0:cf, :].rearrange("p (q g) v -> p q g v", g=G)
                    nc.vector.tensor_mul(fz4, d_z, wt4)
                    nc.vector.tensor_add(fz4, fz4, f0_z)

                    # --- products (y/x rows are supersampled, direct) ---
                    t = wp.tile([P, 32, C], dt.float16, tag="t")
                    nc.vector.tensor_mul(t[:, 0:cf, :], gy[:, 0:cf, 0:C],
                                         gx[:, 0:cf, 0:C])
                    nc.vector.tensor_mul(t[:, 0:cf, :], t[:, 0:cf, :],
                                         fz[:, 0:cf, :])
                    # binary-tree halvings at tensor_tensor 2x rate, then a
                    # short tensor_reduce tail (reduce gets no DVE perf mode)
                    half = C
                    while half >= 12:
                        half //= 2
                        nc.vector.tensor_add(t[:, 0:cf, 0:half],
                                             t[:, 0:cf, 0:half],
                                             t[:, 0:cf, half:2 * half])
                    nc.vector.reduce_sum(out_full[:, fs:fs + cf],
                                         t[:, 0:cf, 0:half], axis=Axis.X)

                nc.sync.dma_start(out_d, out_full[:])

    nc.compile()
    return nc


def _host_prep(in_tensor, line_z, line_y, line_x):
    """Build per-core input maps; returns (in_maps, per-core unsort perms)."""
    pts = np.ascontiguousarray(in_tensor.reshape(-1, 3).astype(np.float32))

    # z table: [f0(96) | delta(96) | pad] rows of 512B
    Lz = np.asarray(line_z, dtype=np.float32)
    z0 = Lz.T                                        # (512, 96)
    z1 = np.concatenate([Lz.T[1:], Lz.T[-1:]], axis=0)
    tbl_z = np.zeros((R, ELEMZ), dtype=np.float16)
    tbl_z[:, 0:C] = z0.astype(np.float16)
    tbl_z[:, C:2 * C] = (z1 - z0).astype(np.float16)

    # y/x tables: 64x supersampled, interpolation baked in, f0-only 256B rows
    def supersample(L):
        Lf = np.asarray(L, dtype=np.float32).T       # (512, 96)
        f0 = Lf
        f1 = np.concatenate([Lf[1:], Lf[-1:]], axis=0)
        r = (np.arange(SS, dtype=np.float32) / SS)[None, :, None]
        fine = f0[:, None, :] * (1.0 - r) + f1[:, None, :] * r
        row = np.zeros((R * SS, SELEM), dtype=np.float16)
        row[:, 0:C] = fine.reshape(R * SS, C).astype(np.float16)
        return row
    tbl_y = supersample(line_y)
    tbl_x = supersample(line_x)

    # per-point indices/weights, axes ordered [z, y, x] = cols [2, 1, 0]
    pos = (pts + 1.0) * 0.5 * (R - 1)
    i0 = np.clip(np.floor(pos), 0, R - 1).astype(np.int32)
    w = (pos - i0).astype(np.float16)
    # supersampled y/x indices (nearest of the 64x grid)
    isup = np.clip(np.round(pos * SS), 0, (R - 1) * SS).astype(np.int32)

    def wrap16(flat):
        """j-ordered descriptor index list -> [16, n/16] band, replicated
        to all 8 16-partition bands."""
        w16 = flat.reshape(-1, 16).T
        return np.ascontiguousarray(np.tile(w16, (8, 1)))

    in_maps = []
    perms = []
    for k in range(N_CORES):
        sl = slice(k * N_CORE, (k + 1) * N_CORE)
        iz = i0[sl, 2]
        iy, ix = isup[sl, 1], isup[sl, 0]
        wz = w[sl, 2]

        # sort by z-index; emit fixed-size groups of G per z-bin (padded)
        order = np.argsort(iz, kind="stable")
        izs = iz[order]
        # position of each sorted point within its z-bin
        binpos = np.arange(N_CORE) - np.searchsorted(izs, izs, side="left")
        ggid = binpos // G                            # group within bin
        key = izs.astype(np.int64) * 4096 + ggid      # global (bin, group)
        uniq, ginv = np.unique(key, return_inverse=True)
        n_groups = len(uniq)
        assert n_groups * G <= S, f"padding overflow: {n_groups * G} > {S}"
        slot_in_g = binpos % G
        # group g occupies partition g%128, free blocks (g//128)*G + m
        part = (ginv % P).astype(np.int32)
        free = ((ginv // P) * G + slot_in_g).astype(np.int32)

        # z-row per group, one descriptor per group, j == g ordering
        zrow = np.zeros(S // G, dtype=np.int16)
        zrow[:n_groups] = (uniq // 4096).astype(np.int16)

        # per-slot w / y / x arrays in (partition, free) layout
        w_arr = np.zeros((P, F), dtype=np.float16)
        iy_arr = np.zeros((P, F), dtype=np.int16)
        ix_arr = np.zeros((P, F), dtype=np.int16)
        w_arr[part, free] = wz[order]
        iy_arr[part, free] = iy[order].astype(np.int16)
        ix_arr[part, free] = ix[order].astype(np.int16)

        # y/x descriptor j = f*128 + p  (desc j lands in out[j%128, j//128])
        in_maps.append({
            "w": w_arr,
            "idxz": wrap16(zrow),
            "idxy": wrap16(iy_arr.T.reshape(-1)),
            "idxx": wrap16(ix_arr.T.reshape(-1)),
            "tblz": tbl_z,
            "tbly": tbl_y,
            "tblx": tbl_x,
        })
        # inverse mapping: sorted order + slot coordinates
        perms.append((order, part, free))
    return in_maps, perms


def _unshard(results, perms):
    outs = []
    for k in range(N_CORES):
        wv = np.asarray(results[k]["out"])           # (P, F)
        order, part, free = perms[k]
        vals = wv[part, free]                        # sorted-point order
        o = np.empty(N_CORE, dtype=np.float32)
        o[order] = vals
        outs.append(o)
    return np.concatenate(outs).reshape(4096, 192).astype(np.float32)


def kernel(in_tensor, line_z, line_y, line_x):
    global _BUILT
    from concourse.bass_utils import run_bass_kernel_spmd

    if _BUILT is None:
        _BUILT = _build_nc()
    nc = _BUILT
    in_maps, perms = _host_prep(np.asarray(in_tensor), np.asarray(line_z),
                                np.asarray(line_y), np.asarray(line_x))
    res = run_bass_kernel_spmd(nc, in_maps, list(range(N_CORES)))
    return _unshard(res.results, perms)


# revision 14
# speedup vs baseline: 1.0364x; 1.0364x over previous
"""CPDecoding (embedding_lookup) Trainium2 kernel, v5.

out[n] = sum_c fz[c,n]*fy[c,n]*fx[c,n], where f* is a 1-D linear
interpolation (grid_sample, align_corners=True) of a (96, 512) line table
at per-point coordinates.

The kernel is DMA-descriptor-bound: every gather descriptor costs
max(elem_bytes*(2 if <512B else 1)/22.5, 7)/16 ns and elem_bytes must be a
multiple of 256, so each per-point fetch costs 22.76/16 ns no matter the
dtype. Strategy (8 cores, data-parallel over the N=4096*192 points):
  - Host: compute (i0, w) per point/axis; sort points by z-index and pack
    8 points per z-table row (one 512B descriptor serves 8 points), padded
    to a fixed slot count.
  - y AND x tables are 64x supersampled fp16 f0-only 256B rows
    (interpolation baked in): one descriptor per point per axis, and the
    x-interp disappears from the device (vs v2: -2 DVE ops, -1 Act op).
  - idx tensors are streamed per chunk (overlapped with gathers) instead
    of an 11us serial prologue; F is trimmed to 784 (the fixed-seed data
    needs 775.4, at +12 sigma of the group-count distribution), cutting 2%
    of all per-point descriptors.
  - The last 16 f-slots run as 2 small chunks and the bulk of the output
    is written back early, so the trailing non-overlapped stretch is
    ~3us instead of ~10us.
  - Per 4096-point chunk: DMA ~12.5us (bound), DVE ~8.7us, Act ~2.8us,
    Pool ~6.1us, all overlapped.
  - Host: unpermute per-point sums to the original order.
y+x supersampling adds ~1.1% rel err (budget 2e-2).
"""

import numpy as np

N_CORES = 8
N_TOTAL = 4096 * 192
N_CORE = N_TOTAL // N_CORES      # 98304 points per core
P = 128                          # partitions
G = 8                            # points per z-row group
F = 784                          # free slots per partition (padded)
S = P * F                        # 100352 padded point slots per core
C = 96                           # components
R = 512                          # table resolution
ELEMZ = 256                      # fp16 elements per z table row (512 bytes)
SS = 64                          # y/x table supersampling factor
SELEM = 128                      # fp16 elements per y/x table row (256 bytes)
# chunk schedule: (f_start, chunk_f); small tail chunks shrink the final
# non-overlapped compute stretch
CHUNKS = [(32 * c, 32) for c in range(24)] + [(768, 8), (776, 8)]

_BUILT = None


def _build_nc():
    """Build the per-core Bass program (SPMD, identical on all cores)."""
    import concourse.bacc as bacc
    import concourse.tile as tile
    from concourse import mybir
    from concourse.library_config import mlp as lib_mlp

    dt = mybir.dt
    Axis = mybir.AxisListType

    nc = bacc.Bacc("TRN2", target_bir_lowering=False, debug=False,
                   num_devices=N_CORES, num_swdge_queues=1)

    # host-prepared inputs (idx wrapped-16 and replicated to all 8 bands --
    # each SWDGE engine reads its own 16-partition band)
    w_d = nc.dram_tensor("w", [P, F], dt.float16, kind="ExternalInput").ap()
    idxz_d = nc.dram_tensor("idxz", [P, F], dt.int16,
                            kind="ExternalInput").ap()
    idxy_d = nc.dram_tensor("idxy", [P, S // 16], dt.int16,
                            kind="ExternalInput").ap()
    idxx_d = nc.dram_tensor("idxx", [P, S // 16], dt.int16,
                            kind="ExternalInput").ap()
    tblz = nc.dram_tensor("tblz", [R, ELEMZ], dt.float16,
                          kind="ExternalInput").ap()
    tbly = nc.dram_tensor("tbly", [R * SS, SELEM], dt.float16,
                          kind="ExternalInput").ap()
    tblx = nc.dram_tensor("tblx", [R * SS, SELEM], dt.float16,
                          kind="ExternalInput").ap()
    out_d = nc.dram_tensor("out", [P, F], dt.float32, kind="ExternalOutput").ap()

    with tile.TileContext(nc) as tc:
        with tc.tile_pool(name="persist", bufs=1) as pp:
            with tc.tile_critical():
                nc.gpsimd.load_library(lib_mlp)
            w_all = pp.tile([P, F], dt.float16, tag="w")
            nc.sync.dma_start(w_all[:], w_d)
            idx_z = pp.tile([P, F], dt.int16, tag="iz")
            nc.sync.dma_start(idx_z[:], idxz_d)
            out_full = pp.tile([P, F], dt.float32, tag="out")

            with (
                tc.tile_pool(name="idx", bufs=3) as ip,
                tc.tile_pool(name="gath", bufs=3) as gp,
                tc.tile_pool(name="work", bufs=3) as wp,
            ):
                for ci, (fs, cf) in enumerate(CHUNKS):
                    npts = P * cf                    # 4096 or 1024
                    ycols = npts // 16               # idx columns (wrapped-16)
                    nz = npts // G                   # z descriptors
                    zcols = nz // 16                 # == cf

                    # --- idx slices, streamed per chunk (overlaps gathers) ---
                    iy_t = ip.tile([P, 256], dt.int16, tag="iy")
                    nc.sync.dma_start(iy_t[:, 0:ycols],
                                      idxy_d[:, fs * 8:fs * 8 + ycols])
                    ix_t = ip.tile([P, 256], dt.int16, tag="ix")
                    nc.sync.dma_start(ix_t[:, 0:ycols],
                                      idxx_d[:, fs * 8:fs * 8 + ycols])

                    # --- gathers (z first: its data is consumed first,
                    # so the post-DMA dependency chain is just t/p/reduce) ---
                    gz = gp.tile([P, 4, ELEMZ], dt.float16, tag="gz")
                    nc.gpsimd.dma_gather(
                        gz[:, 0:nz // P, :], tblz,
                        idx_z[:, fs:fs + zcols],
                        nz, nz, ELEMZ, elem_step=ELEMZ,
                        queue_num=0, single_packet=False)
                    gx = gp.tile([P, 32, SELEM], dt.float16, tag="gx")
                    nc.gpsimd.dma_gather(
                        gx[:, 0:cf, :], tblx, ix_t[:, 0:ycols],
                        npts, npts, SELEM, elem_step=SELEM,
                        queue_num=0, single_packet=False)
                    gy = gp.tile([P, 32, SELEM], dt.float16, tag="gy")
                    nc.gpsimd.dma_gather(
                        gy[:, 0:cf, :], tbly, iy_t[:, 0:ycols],
                        npts, npts, SELEM, elem_step=SELEM,
                        queue_num=0, single_packet=False)

                    # --- z weight broadcast (Act engine) ---
                    wtz = wp.tile([P, 32, C], dt.float16, tag="wtz")
                    nc.scalar.copy(wtz[:, 0:cf, :], w_all[:, fs:fs + cf]
                                   .unsqueeze(2).broadcast_to([P, cf, C]))

                    # --- z interp (rows shared by groups of 8 points) ---
                    zb = nz // P                     # 4 or 1
                    d_z = (gz[:, 0:zb, C:2 * C].unsqueeze(2)
                           .broadcast_to([P, zb, G, C]))
                    f0_z = (gz[:, 0:zb, 0:C].unsqueeze(2)
                            .broadcast_to([P, zb, G, C]))
                    fz = wp.tile([P, 32, C], dt.float16, tag="fz")
                    fz4 = fz[:, 0:cf, :].rearrange("p (q g) v -> p q g v", g=G)
                    wt4 = wtz[:, 0:cf, :].rearrange("p (q g) v -> p q g v", g=G)
                    nc.vector.tensor_mul(fz4, d_z, wt4)
                    nc.vector.tensor_add(fz4, fz4, f0_z)

                    # --- products (y/x rows are supersampled, direct) ---
                    t = wp.tile([P, 32, C], dt.float16, tag="t")
                    nc.vector.tensor_mul(t[:, 0:cf, :], gy[:, 0:cf, 0:C],
                                         gx[:, 0:cf, 0:C])
                    nc.vector.tensor_mul(t[:, 0:cf, :], t[:, 0:cf, :],
                                         fz[:, 0:cf, :])
                    # binary-tree halvings at tensor_tensor 2x rate, then a
                    # short tensor_reduce tail (reduce gets no DVE perf mode)
                    half = C
                    while half >= 12:
                        half //= 2
                        nc.vector.tensor_add(t[:, 0:cf, 0:half],
                                             t[:, 0:cf, 0:half],
                                             t[:, 0:cf, half:2 * half])
                    nc.vector.reduce_sum(out_full[:, fs:fs + cf],
                                         t[:, 0:cf, 0:half], axis=Axis.X)
                    if fs + cf == 768:
                        # bulk writeback overlaps the small tail chunks
                        nc.sync.dma_start(out_d[:, 0:768], out_full[:, 0:768])

                nc.sync.dma_start(out_d[:, 768:F], out_full[:, 768:F])

    nc.compile()
    return nc


def _host_prep(in_tensor, line_z, line_y, line_x):
    """Build per-core input maps; returns (in_maps, per-core unsort perms)."""
    pts = np.ascontiguousarray(in_tensor.reshape(-1, 3).astype(np.float32))

    # z table: [f0(96) | delta(96) | pad] rows of 512B
    Lz = np.asarray(line_z, dtype=np.float32)
    z0 = Lz.T                                        # (512, 96)
    z1 = np.concatenate([Lz.T[1:], Lz.T[-1:]], axis=0)
    tbl_z = np.zeros((R, ELEMZ), dtype=np.float16)
    tbl_z[:, 0:C] = z0.astype(np.float16)
    tbl_z[:, C:2 * C] = (z1 - z0).astype(np.float16)

    # y/x tables: 64x supersampled, interpolation baked in, f0-only 256B rows
    def supersample(L):
        Lf = np.asarray(L, dtype=np.float32).T       # (512, 96)
        f0 = Lf
        f1 = np.concatenate([Lf[1:], Lf[-1:]], axis=0)
        r = (np.arange(SS, dtype=np.float32) / SS)[None, :, None]
        fine = f0[:, None, :] * (1.0 - r) + f1[:, None, :] * r
        row = np.zeros((R * SS, SELEM), dtype=np.float16)
        row[:, 0:C] = fine.reshape(R * SS, C).astype(np.float16)
        return row
    tbl_y = supersample(line_y)
    tbl_x = supersample(line_x)

    # per-point indices/weights, axes ordered [z, y, x] = cols [2, 1, 0]
    pos = (pts + 1.0) * 0.5 * (R - 1)
    i0 = np.clip(np.floor(pos), 0, R - 1).astype(np.int32)
    w = (pos - i0).astype(np.float16)
    # supersampled y/x indices (nearest of the 64x grid)
    isup = np.clip(np.round(pos * SS), 0, (R - 1) * SS).astype(np.int32)

    def wrap16(flat):
        """j-ordered descriptor index list -> [16, n/16] band, replicated
        to all 8 16-partition bands."""
        w16 = flat.reshape(-1, 16).T
        return np.ascontiguousarray(np.tile(w16, (8, 1)))

    in_maps = []
    perms = []
    for k in range(N_CORES):
        sl = slice(k * N_CORE, (k + 1) * N_CORE)
        iz = i0[sl, 2]
        iy, ix = isup[sl, 1], isup[sl, 0]
        wz = w[sl, 2]

        # sort by z-index; emit fixed-size groups of G per z-bin (padded)
        order = np.argsort(iz, kind="stable")
        izs = iz[order]
        # position of each sorted point within its z-bin
        binpos = np.arange(N_CORE) - np.searchsorted(izs, izs, side="left")
        ggid = binpos // G                            # group within bin
        key = izs.astype(np.int64) * 4096 + ggid      # global (bin, group)
        uniq, ginv = np.unique(key, return_inverse=True)
        n_groups = len(uniq)
        assert n_groups * G <= S, f"padding overflow: {n_groups * G} > {S}"
        slot_in_g = binpos % G
        # group g occupies partition g%128, free blocks (g//128)*G + m
        part = (ginv % P).astype(np.int32)
        free = ((ginv // P) * G + slot_in_g).astype(np.int32)

        # z-row per group, one descriptor per group, j == g ordering
        zrow = np.zeros(S // G, dtype=np.int16)
        zrow[:n_groups] = (uniq // 4096).astype(np.int16)

        # per-slot w / y / x arrays in (partition, free) layout
        w_arr = np.zeros((P, F), dtype=np.float16)
        iy_arr = np.zeros((P, F), dtype=np.int16)
        ix_arr = np.zeros((P, F), dtype=np.int16)
        w_arr[part, free] = wz[order]
        iy_arr[part, free] = iy[order].astype(np.int16)
        ix_arr[part, free] = ix[order].astype(np.int16)

        # y/x descriptor j = f*128 + p  (desc j lands in out[j%128, j//128])
        in_maps.append({
            "w": w_arr,
            "idxz": wrap16(zrow),
            "idxy": wrap16(iy_arr.T.reshape(-1)),
            "idxx": wrap16(ix_arr.T.reshape(-1)),
            "tblz": tbl_z,
            "tbly": tbl_y,
            "tblx": tbl_x,
        })
        # inverse mapping: sorted order + slot coordinates
        perms.append((order, part, free))
    return in_maps, perms


def _unshard(results, perms):
    outs = []
    for k in range(N_CORES):
        wv = np.asarray(results[k]["out"])           # (P, F)
        order, part, free = perms[k]
        vals = wv[part, free]                        # sorted-point order
        o = np.empty(N_CORE, dtype=np.float32)
        o[order] = vals
        outs.append(o)
    return np.concatenate(outs).reshape(4096, 192).astype(np.float32)


def kernel(in_tensor, line_z, line_y, line_x):
    global _BUILT
    from concourse.bass_utils import run_bass_kernel_spmd

    if _BUILT is None:
        _BUILT = _build_nc()
    nc = _BUILT
    in_maps, perms = _host_prep(np.asarray(in_tensor), np.asarray(line_z),
                                np.asarray(line_y), np.asarray(line_x))
    res = run_bass_kernel_spmd(nc, in_maps, list(range(N_CORES)))
    return _unshard(res.results, perms)
